# revision 1
# baseline (speedup 1.0000x reference)
"""AttentionPooling (segment softmax + weighted segment sum) on 8 trn2 cores.

Math (per graph g): out[g] = sum_n softmax_g(s)_n * x[n] over nodes n with
batch[n] == g, where s = tanh(x @ W1 + b1) @ W2 + b2.

Key observations:
  * |s| <= ||W2||_1 + |b2| ~= 11.3 (tanh output bounded by 1), so exp(s)
    cannot overflow fp32 -> the segment-max subtraction of the reference is
    unnecessary; we accumulate unnormalized exp(s)*x and exp(s) and divide
    once at the end.
  * batch is sorted, so sharding by graph (128 graphs per core) gives each
    core one contiguous node range: pure data parallel, no collectives.
  * The segment sum is a matmul with a one-hot(weighted) matrix:
    out[g, d] = sum_n S[n, g] * x_aug[n, d],  S[n, g] = e_n * (batch[n]==g),
    which maps perfectly to TensorE with 128 graphs == 128 partitions.
  * TensorE contracts over the partition dim, so the MLP needs x with hidden
    on partitions (x_T) while pooling needs nodes on partitions (x_aug).
    On-chip transposes would cost more than streaming both copies from HBM
    in bf16 (2 x 2 bytes == 1 x fp32 read).
"""

import sys
from contextlib import ExitStack

import numpy as np

for _p in ("/opt/trn_rl_repo",):
    if _p not in sys.path:
        sys.path.insert(0, _p)

import ml_dtypes

import concourse.bass as bass
import concourse.bacc as bacc
import concourse.tile as tile
from concourse import mybir

N_NODES = 500_000
HIDDEN = 256
NUM_GRAPHS = 1024
N_CORES = 8
G_LOC = NUM_GRAPHS // N_CORES  # 128 graphs per core == PSUM partition dim
H = HIDDEN // 2  # 128 hidden units in the attention MLP
BLK = 128  # nodes per block (matmul contraction tile)
NBPC = 4  # blocks per chunk
CH = BLK * NBPC  # 512 nodes per compute chunk (one PSUM bank at fp32)
CPS = 4  # compute chunks per DMA super-chunk
SUP = CH * CPS  # 2048 nodes per DMA (~1 MB per stream -> efficient descriptors)
BF16 = mybir.dt.bfloat16
F32 = mybir.dt.float32

_PROGRAM_CACHE: dict[int, bass.Bass] = {}


def build_program(n_pad: int, repeats: int = 1, ablate: str = "") -> bass.Bass:
    """repeats > 1 re-runs the whole accumulation loop; numerators and
    denominators both scale by `repeats`, so the output is unchanged --
    used to measure per-pass hardware time by slope.

    ablate (timing experiments only, wrong results): "no_xaug" drops the
    x_aug DMA stream (pool matmul streams from the xt tile instead);
    "no_mlp" drops the MLP matmuls + tanh (scores read the xt tile)."""
    assert n_pad % SUP == 0
    nblk = n_pad // BLK
    nsup = n_pad // SUP

    nc = bacc.Bacc("TRN2")
    # host-swizzled so each super-chunk DMA reads one contiguous ~8KB run per
    # partition: xaug[s, p, b, f] = [x | 1.0][s*SUP + b*BLK + p, f]
    xaug = nc.dram_tensor(
        "xaug", [nsup, BLK, NBPC * CPS, HIDDEN + 1], BF16, kind="ExternalInput"
    )
    # xT[s, p, j, n] = x[s*SUP + n, BLK*j + p]
    xT = nc.dram_tensor("xT", [nsup, BLK, 2, SUP], BF16, kind="ExternalInput")
    bcols = nc.dram_tensor("bcols", [BLK, nblk], F32, kind="ExternalInput")
    w1 = nc.dram_tensor("w1", [HIDDEN, H], BF16, kind="ExternalInput")
    w2 = nc.dram_tensor("w2", [H, 1], BF16, kind="ExternalInput")
    b1 = nc.dram_tensor("b1", [H, 1], F32, kind="ExternalInput")
    b2 = nc.dram_tensor("b2", [BLK, 1], F32, kind="ExternalInput")
    out = nc.dram_tensor("out", [G_LOC, HIDDEN], F32, kind="ExternalOutput")


    with tile.TileContext(nc) as tc, ExitStack() as ctx:
        singles = ctx.enter_context(tc.tile_pool(name="singles", bufs=1))
        xa_pool = ctx.enter_context(tc.tile_pool(name="xa", bufs=3))
        xt_pool = ctx.enter_context(tc.tile_pool(name="xt", bufs=3))
        tt_pool = ctx.enter_context(tc.tile_pool(name="tt", bufs=4))
        st_pool = ctx.enter_context(tc.tile_pool(name="st", bufs=8))
        e_pool = ctx.enter_context(tc.tile_pool(name="e", bufs=4))
        hp_pool = ctx.enter_context(tc.tile_pool(name="hp", bufs=3, space="PSUM"))
        sp_pool = ctx.enter_context(tc.tile_pool(name="sp", bufs=3, space="PSUM"))
        acc_pool = ctx.enter_context(tc.tile_pool(name="acc", bufs=1, space="PSUM"))

        w1_sb = singles.tile([BLK, 2, H], BF16)
        nc.sync.dma_start(out=w1_sb[:, 0, :], in_=w1[0:BLK, :])
        nc.sync.dma_start(out=w1_sb[:, 1, :], in_=w1[BLK : 2 * BLK, :])
        w2_sb = singles.tile([H, 1], BF16)
        nc.sync.dma_start(out=w2_sb, in_=w2[:, :])
        b1_sb = singles.tile([H, 1], F32)
        nc.sync.dma_start(out=b1_sb, in_=b1[:, :])
        b2_sb = singles.tile([BLK, 1], F32)
        nc.sync.dma_start(out=b2_sb, in_=b2[:, :])
        bc_sb = singles.tile([BLK, nblk], F32)
        nc.sync.dma_start(out=bc_sb, in_=bcols[:, :])
        iota_sb = singles.tile([BLK, G_LOC], F32)
        nc.gpsimd.iota(
            out=iota_sb,
            pattern=[[1, G_LOC]],
            base=0,
            channel_multiplier=0,
            allow_small_or_imprecise_dtypes=True,
        )

        acc = acc_pool.tile([G_LOC, HIDDEN + 1], F32)

        for s_iter in range(nsup * repeats):
            rep, s = divmod(s_iter, nsup)
            if ablate != "no_xaug":
                xa = xa_pool.tile([BLK, NBPC * CPS, HIDDEN + 1], BF16)
                nc.sync.dma_start(out=xa, in_=xaug[s])
            xt = xt_pool.tile([BLK, 2, SUP], BF16)
            nc.sync.dma_start(out=xt, in_=xT[s])

            for q in range(CPS):
                if ablate != "no_mlp":
                    hp = hp_pool.tile([H, CH], F32)
                    nc.tensor.matmul(
                        hp,
                        lhsT=w1_sb[:, 0, :],
                        rhs=xt[:, 0, q * CH : (q + 1) * CH],
                        start=True,
                        stop=False,
                    )
                    nc.tensor.matmul(
                        hp,
                        lhsT=w1_sb[:, 1, :],
                        rhs=xt[:, 1, q * CH : (q + 1) * CH],
                        start=False,
                        stop=True,
                    )

                    tt = tt_pool.tile([H, CH], BF16)
                    nc.scalar.activation(
                        out=tt,
                        in_=hp,
                        func=mybir.ActivationFunctionType.Tanh,
                        bias=b1_sb,
                    )
                else:
                    tt = xt[:, 0, q * CH : (q + 1) * CH]

                sp = sp_pool.tile([BLK, NBPC], F32)
                for b in range(NBPC):
                    nc.tensor.matmul(
                        sp[:, b : b + 1],
                        lhsT=tt[:, b * BLK : (b + 1) * BLK],
                        rhs=w2_sb,
                        start=True,
                        stop=True,
                    )

                ee = e_pool.tile([BLK, NBPC], F32)
                nc.scalar.activation(
                    out=ee, in_=sp, func=mybir.ActivationFunctionType.Exp, bias=b2_sb
                )

                for b in range(NBPC):
                    st = st_pool.tile([BLK, G_LOC], BF16, tag="st")
                    j = (s * CPS + q) * NBPC + b
                    nc.vector.tensor_scalar(
                        out=st,
                        in0=iota_sb,
                        scalar1=bc_sb[:, j : j + 1],
                        scalar2=ee[:, b : b + 1],
                        op0=mybir.AluOpType.is_equal,
                        op1=mybir.AluOpType.mult,
                    )
                    pool_rhs = (
                        xa[:, q * NBPC + b, :]
                        if ablate != "no_xaug"
                        else xt[:, 0, 0 : HIDDEN + 1]
                    )
                    nc.tensor.matmul(
                        acc,
                        lhsT=st,
                        rhs=pool_rhs,
                        start=(rep == 0 and s == 0 and q == 0 and b == 0),
                        stop=(
                            rep == repeats - 1
                            and s == nsup - 1
                            and q == CPS - 1
                            and b == NBPC - 1
                        ),
                    )

        denom = singles.tile([G_LOC, 1], F32)
        nc.vector.tensor_scalar_max(out=denom, in0=acc[:, HIDDEN : HIDDEN + 1], scalar1=1e-30)
        rdenom = singles.tile([G_LOC, 1], F32)
        nc.vector.reciprocal(out=rdenom, in_=denom)
        out_sb = singles.tile([G_LOC, HIDDEN], F32)
        nc.vector.tensor_scalar_mul(out=out_sb, in0=acc[:, 0:HIDDEN], scalar1=rdenom)
        nc.sync.dma_start(out=out[:, :], in_=out_sb)

    nc.finalize()
    return nc


def make_in_maps(x, batch, W1, b1, W2, b2):
    """Shard by graph (128 contiguous graphs per core), pad node counts to a
    common multiple of CH, and lay out the per-core device arrays."""
    x = np.asarray(x, dtype=np.float32)
    batch = np.asarray(batch)
    bounds = np.searchsorted(batch, np.arange(0, NUM_GRAPHS + 1, G_LOC))
    n_loc_max = int(np.diff(bounds).max())
    n_pad = max(SUP, ((n_loc_max + SUP - 1) // SUP) * SUP)

    w1_bf = np.asarray(W1, np.float32).astype(ml_dtypes.bfloat16)
    w2_bf = np.asarray(W2, np.float32).reshape(H, 1).astype(ml_dtypes.bfloat16)
    b1_f = np.asarray(b1, np.float32).reshape(H, 1)
    b2_f = np.full((BLK, 1), np.float32(np.asarray(b2).reshape(-1)[0]), np.float32)

    in_maps = []
    for c in range(N_CORES):
        s, e = int(bounds[c]), int(bounds[c + 1])
        nloc = e - s
        xs = x[s:e]
        nsup = n_pad // SUP
        nb = NBPC * CPS
        xa = np.zeros((n_pad, HIDDEN + 1), ml_dtypes.bfloat16)
        xa[:nloc, :HIDDEN] = xs
        xa[:nloc, HIDDEN] = 1.0
        # [s*SUP + b*BLK + p, f] -> [s, p, b, f]
        xa = np.ascontiguousarray(
            xa.reshape(nsup, nb, BLK, HIDDEN + 1).transpose(0, 2, 1, 3)
        )
        # [s, p, j, n] = x[s*SUP + n, BLK*j + p]
        xT = np.zeros((HIDDEN, n_pad), ml_dtypes.bfloat16)
        xT[:, :nloc] = xs.T
        xT = np.ascontiguousarray(
            xT.reshape(2, BLK, nsup, SUP).transpose(2, 1, 0, 3)
        )
        bl = np.full((n_pad,), -1.0, np.float32)
        bl[:nloc] = batch[s:e].astype(np.float32) - np.float32(c * G_LOC)
        bcols = np.ascontiguousarray(bl.reshape(n_pad // BLK, BLK).T)
        in_maps.append(
            {
                "xaug": xa,
                "xT": xT,
                "bcols": bcols,
                "w1": w1_bf,
                "w2": w2_bf,
                "b1": b1_f,
                "b2": b2_f,
            }
        )
    return in_maps, n_pad


def kernel(x, batch, W1, b1, W2, b2):
    from concourse.bass_utils import run_bass_kernel_spmd

    in_maps, n_pad = make_in_maps(x, batch, W1, b1, W2, b2)
    nc = _PROGRAM_CACHE.get(n_pad)
    if nc is None:
        nc = build_program(n_pad)
        _PROGRAM_CACHE[n_pad] = nc
    res = run_bass_kernel_spmd(nc, in_maps, list(range(N_CORES)))
    return np.concatenate([res.results[c]["out"] for c in range(N_CORES)], axis=0)



# revision 2
# speedup vs baseline: 2183.9039x; 2183.9039x over previous
"""AttentionPooling (segment softmax + weighted segment sum) on 8 trn2 cores.

Math (per graph g): out[g] = sum_n softmax_g(s)_n * x[n] over nodes n with
batch[n] == g, where s = tanh(x @ W1 + b1) @ W2 + b2.

Key observations:
  * |s| <= ||W2||_1 + |b2| ~= 11.3 (tanh output bounded by 1), so exp(s)
    cannot overflow fp32 -> the segment-max subtraction of the reference is
    unnecessary; we accumulate unnormalized exp(s)*x and exp(s) and divide
    once at the end.
  * batch is sorted, so sharding by graph (128 graphs per core) gives each
    core one contiguous node range: pure data parallel, no collectives.
  * The segment sum is a matmul with a one-hot(weighted) matrix:
    out[g, d] = sum_n S[n, g] * x_aug[n, d],  S[n, g] = e_n * (batch[n]==g),
    which maps perfectly to TensorE with 128 graphs == 128 partitions.
  * TensorE contracts over the partition dim, so the MLP needs x with hidden
    on partitions (x_T) while pooling needs nodes on partitions (x_aug).
    Streaming both layouts from HBM costs 2x the bytes; instead we stream
    only x_T (bf16) and produce x_aug on-chip with X-Bar transpose DMAs
    (SBUF->SBUF, ~300 GB/s, no HBM traffic), halving HBM pressure.
"""

import sys
from contextlib import ExitStack

import numpy as np

for _p in ("/opt/trn_rl_repo",):
    if _p not in sys.path:
        sys.path.insert(0, _p)

import ml_dtypes

import concourse.bass as bass
import concourse.bacc as bacc
import concourse.tile as tile
from concourse import mybir

N_NODES = 500_000
HIDDEN = 256
NUM_GRAPHS = 1024
N_CORES = 8
G_LOC = NUM_GRAPHS // N_CORES  # 128 graphs per core == PSUM partition dim
H = HIDDEN // 2  # 128 hidden units in the attention MLP
BLK = 128  # nodes per block (matmul contraction tile)
NBPC = 4  # blocks per chunk
CH = BLK * NBPC  # 512 nodes per compute chunk (one PSUM bank at fp32)
CPS = 4  # compute chunks per DMA super-chunk
SUP = CH * CPS  # 2048 nodes per DMA (~1 MB per stream -> efficient descriptors)
NB = NBPC * CPS  # blocks per super-chunk
BF16 = mybir.dt.bfloat16
F32 = mybir.dt.float32

_PROGRAM_CACHE: dict[int, bass.Bass] = {}


def build_program(n_pad: int, repeats: int = 1, ablate: str = "") -> bass.Bass:
    """repeats > 1 re-runs the whole accumulation loop; numerators and
    denominators both scale by `repeats`, so the output is unchanged --
    used to measure per-pass hardware time by slope.

    ablate (timing experiments only, wrong results): "no_tr" drops the
    on-chip transposes (pool matmul reads whatever is in the xa tile);
    "no_mlp" drops the MLP matmuls + tanh (scores read the xt tile)."""
    assert n_pad % SUP == 0
    nblk = n_pad // BLK
    nsup = n_pad // SUP

    nc = bacc.Bacc("TRN2")
    # xT[s, p, j, n] = x[s*SUP + n, BLK*j + p]
    xT = nc.dram_tensor("xT", [nsup, BLK, 2, SUP], BF16, kind="ExternalInput")
    bcols = nc.dram_tensor("bcols", [BLK, nblk], F32, kind="ExternalInput")
    w1 = nc.dram_tensor("w1", [HIDDEN, H], BF16, kind="ExternalInput")
    w2 = nc.dram_tensor("w2", [H, 1], BF16, kind="ExternalInput")
    b1 = nc.dram_tensor("b1", [H, 1], F32, kind="ExternalInput")
    b2 = nc.dram_tensor("b2", [BLK, 1], F32, kind="ExternalInput")
    out = nc.dram_tensor("out", [G_LOC, HIDDEN], F32, kind="ExternalOutput")

    with tile.TileContext(nc) as tc, ExitStack() as ctx:
        singles = ctx.enter_context(tc.tile_pool(name="singles", bufs=1))
        xa_pool = ctx.enter_context(tc.tile_pool(name="xa", bufs=3))
        xt_pool = ctx.enter_context(tc.tile_pool(name="xt", bufs=3))
        tt_pool = ctx.enter_context(tc.tile_pool(name="tt", bufs=4))
        st_pool = ctx.enter_context(tc.tile_pool(name="st", bufs=8))
        e_pool = ctx.enter_context(tc.tile_pool(name="e", bufs=4))
        hp_pool = ctx.enter_context(tc.tile_pool(name="hp", bufs=3, space="PSUM"))
        sp_pool = ctx.enter_context(tc.tile_pool(name="sp", bufs=3, space="PSUM"))
        acc_pool = ctx.enter_context(tc.tile_pool(name="acc", bufs=1, space="PSUM"))

        w1_sb = singles.tile([BLK, 2, H], BF16)
        nc.sync.dma_start(out=w1_sb[:, 0, :], in_=w1[0:BLK, :])
        nc.sync.dma_start(out=w1_sb[:, 1, :], in_=w1[BLK : 2 * BLK, :])
        w2_sb = singles.tile([H, 1], BF16)
        nc.sync.dma_start(out=w2_sb, in_=w2[:, :])
        b1_sb = singles.tile([H, 1], F32)
        nc.sync.dma_start(out=b1_sb, in_=b1[:, :])
        b2_sb = singles.tile([BLK, 1], F32)
        nc.sync.dma_start(out=b2_sb, in_=b2[:, :])
        bc_sb = singles.tile([BLK, nblk], F32)
        nc.sync.dma_start(out=bc_sb, in_=bcols[:, :])
        iota_sb = singles.tile([BLK, G_LOC], F32)
        nc.gpsimd.iota(
            out=iota_sb,
            pattern=[[1, G_LOC]],
            base=0,
            channel_multiplier=0,
            allow_small_or_imprecise_dtypes=True,
        )

        acc = acc_pool.tile([G_LOC, HIDDEN + 1], F32)

        for s_iter in range(nsup * repeats):
            rep, s = divmod(s_iter, nsup)
            xt = xt_pool.tile([BLK, 2, SUP], BF16)
            nc.sync.dma_start(out=xt, in_=xT[s])

            # x_aug [node-in-block, block, hidden|1]: produced on-chip from
            # xt by X-Bar transpose (SBUF->SBUF); col 256 is the ones column
            # for the softmax denominator.
            xa = xa_pool.tile([BLK, NB, HIDDEN + 1], BF16)
            if ablate != "no_tr":
                # out[p][b][d] = in[d, b*128 + p]  (interp: reshape+T)
                nc.scalar.dma_start(
                    out=xa[:, :, 0:BLK], in_=xt[:, 0, :], transpose=True
                )
                nc.scalar.dma_start(
                    out=xa[:, :, BLK : 2 * BLK], in_=xt[:, 1, :], transpose=True
                )
            nc.vector.memset(xa[:, :, HIDDEN : HIDDEN + 1], 1.0)

            for q in range(CPS):
                if ablate != "no_mlp":
                    hp = hp_pool.tile([H, CH], F32)
                    nc.tensor.matmul(
                        hp,
                        lhsT=w1_sb[:, 0, :],
                        rhs=xt[:, 0, q * CH : (q + 1) * CH],
                        start=True,
                        stop=False,
                    )
                    nc.tensor.matmul(
                        hp,
                        lhsT=w1_sb[:, 1, :],
                        rhs=xt[:, 1, q * CH : (q + 1) * CH],
                        start=False,
                        stop=True,
                    )

                    tt = tt_pool.tile([H, CH], BF16)
                    nc.scalar.activation(
                        out=tt,
                        in_=hp,
                        func=mybir.ActivationFunctionType.Tanh,
                        bias=b1_sb,
                    )
                else:
                    tt = xt[:, 0, q * CH : (q + 1) * CH]

                sp = sp_pool.tile([BLK, NBPC], F32)
                for b in range(NBPC):
                    nc.tensor.matmul(
                        sp[:, b : b + 1],
                        lhsT=tt[:, b * BLK : (b + 1) * BLK],
                        rhs=w2_sb,
                        start=True,
                        stop=True,
                    )

                ee = e_pool.tile([BLK, NBPC], F32)
                nc.scalar.activation(
                    out=ee, in_=sp, func=mybir.ActivationFunctionType.Exp, bias=b2_sb
                )

                for b in range(NBPC):
                    st = st_pool.tile([BLK, G_LOC], BF16, tag="st")
                    j = (s * CPS + q) * NBPC + b
                    nc.vector.tensor_scalar(
                        out=st,
                        in0=iota_sb,
                        scalar1=bc_sb[:, j : j + 1],
                        scalar2=ee[:, b : b + 1],
                        op0=mybir.AluOpType.is_equal,
                        op1=mybir.AluOpType.mult,
                    )
                    nc.tensor.matmul(
                        acc,
                        lhsT=st,
                        rhs=xa[:, q * NBPC + b, :],
                        start=(rep == 0 and s == 0 and q == 0 and b == 0),
                        stop=(
                            rep == repeats - 1
                            and s == nsup - 1
                            and q == CPS - 1
                            and b == NBPC - 1
                        ),
                    )

        denom = singles.tile([G_LOC, 1], F32)
        nc.vector.tensor_scalar_max(out=denom, in0=acc[:, HIDDEN : HIDDEN + 1], scalar1=1e-30)
        rdenom = singles.tile([G_LOC, 1], F32)
        nc.vector.reciprocal(out=rdenom, in_=denom)
        out_sb = singles.tile([G_LOC, HIDDEN], F32)
        nc.vector.tensor_scalar_mul(out=out_sb, in0=acc[:, 0:HIDDEN], scalar1=rdenom)
        nc.sync.dma_start(out=out[:, :], in_=out_sb)

    nc.finalize()
    return nc


def make_in_maps(x, batch, W1, b1, W2, b2):
    """Shard by graph (128 contiguous graphs per core), pad node counts to a
    common multiple of SUP, and lay out the per-core device arrays."""
    x = np.asarray(x, dtype=np.float32)
    batch = np.asarray(batch)
    bounds = np.searchsorted(batch, np.arange(0, NUM_GRAPHS + 1, G_LOC))
    n_loc_max = int(np.diff(bounds).max())
    n_pad = max(SUP, ((n_loc_max + SUP - 1) // SUP) * SUP)

    w1_bf = np.asarray(W1, np.float32).astype(ml_dtypes.bfloat16)
    w2_bf = np.asarray(W2, np.float32).reshape(H, 1).astype(ml_dtypes.bfloat16)
    b1_f = np.asarray(b1, np.float32).reshape(H, 1)
    b2_f = np.full((BLK, 1), np.float32(np.asarray(b2).reshape(-1)[0]), np.float32)

    in_maps = []
    for c in range(N_CORES):
        s, e = int(bounds[c]), int(bounds[c + 1])
        nloc = e - s
        xs = x[s:e]
        nsup = n_pad // SUP
        # [s, p, j, n] = x[s*SUP + n, BLK*j + p]
        xT = np.zeros((HIDDEN, n_pad), ml_dtypes.bfloat16)
        xT[:, :nloc] = xs.T
        xT = np.ascontiguousarray(
            xT.reshape(2, BLK, nsup, SUP).transpose(2, 1, 0, 3)
        )
        bl = np.full((n_pad,), -1.0, np.float32)
        bl[:nloc] = batch[s:e].astype(np.float32) - np.float32(c * G_LOC)
        bcols = np.ascontiguousarray(bl.reshape(n_pad // BLK, BLK).T)
        in_maps.append(
            {
                "xT": xT,
                "bcols": bcols,
                "w1": w1_bf,
                "w2": w2_bf,
                "b1": b1_f,
                "b2": b2_f,
            }
        )
    return in_maps, n_pad


def kernel(x, batch, W1, b1, W2, b2):
    from concourse.bass_utils import run_bass_kernel_spmd

    in_maps, n_pad = make_in_maps(x, batch, W1, b1, W2, b2)
    nc = _PROGRAM_CACHE.get(n_pad)
    if nc is None:
        nc = build_program(n_pad)
        _PROGRAM_CACHE[n_pad] = nc
    res = run_bass_kernel_spmd(nc, in_maps, list(range(N_CORES)))
    return np.concatenate([res.results[c]["out"] for c in range(N_CORES)], axis=0)


# revision 9
# speedup vs baseline: 2360.7069x; 1.0810x over previous
"""AttentionPooling (segment softmax + weighted segment sum) on 8 trn2 cores.

Math (per graph g): out[g] = sum_n softmax_g(s)_n * x[n] over nodes n with
batch[n] == g, where s = tanh(x @ W1 + b1) @ W2 + b2.

Key observations:
  * |s| <= ||W2||_1 + |b2| ~= 11.3 (tanh output bounded by 1), so exp(s)
    cannot overflow fp32 -> the segment-max subtraction of the reference is
    unnecessary; we accumulate unnormalized exp(s)*x and exp(s) and divide
    once at the end.
  * batch is sorted, so sharding by graph (128 graphs per core) gives each
    core one contiguous node range: pure data parallel, no collectives.
  * The segment sum is a matmul with a one-hot(weighted) matrix:
    out[g, d] = sum_n S[n, g] * x_aug[n, d],  S[n, g] = e_n * (batch[n]==g),
    which maps perfectly to TensorE with 128 graphs == 128 partitions.
  * TensorE contracts over the partition dim, so the MLP needs x with hidden
    on partitions (x_T) while pooling needs nodes on partitions (x_aug).
    On-chip transposes would cost more than streaming both copies from HBM
    in bf16 (2 x 2 bytes == 1 x fp32 read).
"""

import sys
from contextlib import ExitStack

import numpy as np

for _p in ("/opt/trn_rl_repo",):
    if _p not in sys.path:
        sys.path.insert(0, _p)

import ml_dtypes

import concourse.bass as bass
import concourse.bacc as bacc
import concourse.tile as tile
from concourse import mybir

N_NODES = 500_000
HIDDEN = 256
NUM_GRAPHS = 1024
N_CORES = 8
G_LOC = NUM_GRAPHS // N_CORES  # 128 graphs per core == PSUM partition dim
H = HIDDEN // 2  # 128 hidden units in the attention MLP
BLK = 128  # nodes per block (matmul contraction tile)
NBPC = 4  # blocks per chunk
CH = BLK * NBPC  # 512 nodes per compute chunk (one PSUM bank at fp32)
CPS = 4  # compute chunks per DMA super-chunk
SUP = CH * CPS  # 2048 nodes per DMA (~1 MB per stream -> efficient descriptors)
BF16 = mybir.dt.bfloat16
FP8 = mybir.dt.float8e4
F32 = mybir.dt.float32

_PROGRAM_CACHE: dict[int, bass.Bass] = {}


def build_program(n_pad: int, repeats: int = 1, ablate: str = "") -> bass.Bass:
    """repeats > 1 re-runs the whole accumulation loop; numerators and
    denominators both scale by `repeats`, so the output is unchanged --
    used to measure per-pass hardware time by slope.

    ablate (timing experiments only, wrong results): "no_xaug" drops the
    x_aug DMA stream (pool matmul streams from the xt tile instead);
    "no_mlp" drops the MLP matmuls + tanh (scores read the xt tile)."""
    assert n_pad % SUP == 0
    nblk = n_pad // BLK
    nsup = n_pad // SUP

    nc = bacc.Bacc("TRN2")
    # host-swizzled so each super-chunk DMA reads one contiguous ~8KB run per
    # partition: xaug[s, p, b, f] = [x | 1.0][s*SUP + b*BLK + p, f]
    xaug = nc.dram_tensor(
        "xaug", [nsup, BLK, NBPC * CPS, HIDDEN + 1], BF16, kind="ExternalInput"
    )
    # xT[s, p, j, n] = x[s*SUP + n, BLK*j + p]; fp8 — only feeds the score
    # MLP, where quantization error perturbs softmax weights mildly, while
    # the pooled values (xaug) stay bf16. 25% less HBM traffic.
    xT = nc.dram_tensor("xT", [nsup, BLK, 2, SUP], FP8, kind="ExternalInput")
    bcols = nc.dram_tensor("bcols", [BLK, nblk], F32, kind="ExternalInput")
    w1 = nc.dram_tensor("w1", [HIDDEN, H], FP8, kind="ExternalInput")
    w2 = nc.dram_tensor("w2", [H, 1], BF16, kind="ExternalInput")
    b1 = nc.dram_tensor("b1", [H, 1], F32, kind="ExternalInput")
    b2 = nc.dram_tensor("b2", [BLK, 1], F32, kind="ExternalInput")
    out = nc.dram_tensor("out", [G_LOC, HIDDEN], F32, kind="ExternalOutput")


    with tile.TileContext(nc) as tc, ExitStack() as ctx:
        singles = ctx.enter_context(tc.tile_pool(name="singles", bufs=1))
        xa_pool = ctx.enter_context(tc.tile_pool(name="xa", bufs=3))
        xt_pool = ctx.enter_context(tc.tile_pool(name="xt", bufs=3))
        tt_pool = ctx.enter_context(tc.tile_pool(name="tt", bufs=4))
        st_pool = ctx.enter_context(tc.tile_pool(name="st", bufs=8))
        e_pool = ctx.enter_context(tc.tile_pool(name="e", bufs=4))
        hp_pool = ctx.enter_context(tc.tile_pool(name="hp", bufs=3, space="PSUM"))
        sp_pool = ctx.enter_context(tc.tile_pool(name="sp", bufs=3, space="PSUM"))
        acc_pool = ctx.enter_context(tc.tile_pool(name="acc", bufs=1, space="PSUM"))

        w1_sb = singles.tile([BLK, 2, H], FP8)
        nc.sync.dma_start(out=w1_sb[:, 0, :], in_=w1[0:BLK, :])
        nc.sync.dma_start(out=w1_sb[:, 1, :], in_=w1[BLK : 2 * BLK, :])
        w2_sb = singles.tile([H, 1], BF16)
        nc.sync.dma_start(out=w2_sb, in_=w2[:, :])
        b1_sb = singles.tile([H, 1], F32)
        nc.sync.dma_start(out=b1_sb, in_=b1[:, :])
        b2_sb = singles.tile([BLK, 1], F32)
        nc.sync.dma_start(out=b2_sb, in_=b2[:, :])
        bc_sb = singles.tile([BLK, nblk], F32)
        nc.sync.dma_start(out=bc_sb, in_=bcols[:, :])
        iota_sb = singles.tile([BLK, G_LOC], F32)
        nc.gpsimd.iota(
            out=iota_sb,
            pattern=[[1, G_LOC]],
            base=0,
            channel_multiplier=0,
            allow_small_or_imprecise_dtypes=True,
        )

        acc = acc_pool.tile([G_LOC, HIDDEN + 1], F32)

        for s_iter in range(nsup * repeats):
            rep, s = divmod(s_iter, nsup)
            if ablate != "no_xaug":
                xa = xa_pool.tile([BLK, NBPC * CPS, HIDDEN + 1], BF16)
                nc.sync.dma_start(out=xa, in_=xaug[s])
            xt = xt_pool.tile([BLK, 2, SUP], FP8)
            nc.sync.dma_start(out=xt, in_=xT[s])

            for q in range(CPS):
                if ablate != "no_mlp":
                    hp = hp_pool.tile([H, CH], F32)
                    nc.tensor.matmul(
                        hp,
                        lhsT=w1_sb[:, 0, :],
                        rhs=xt[:, 0, q * CH : (q + 1) * CH],
                        start=True,
                        stop=False,
                    )
                    nc.tensor.matmul(
                        hp,
                        lhsT=w1_sb[:, 1, :],
                        rhs=xt[:, 1, q * CH : (q + 1) * CH],
                        start=False,
                        stop=True,
                    )

                    tt = tt_pool.tile([H, CH], BF16)
                    nc.scalar.activation(
                        out=tt,
                        in_=hp,
                        func=mybir.ActivationFunctionType.Tanh,
                        bias=b1_sb,
                    )
                else:
                    tt = xt[:, 0, q * CH : (q + 1) * CH]

                sp = sp_pool.tile([BLK, NBPC], F32)
                for b in range(NBPC):
                    nc.tensor.matmul(
                        sp[:, b : b + 1],
                        lhsT=tt[:, b * BLK : (b + 1) * BLK],
                        rhs=w2_sb,
                        start=True,
                        stop=True,
                    )

                ee = e_pool.tile([BLK, NBPC], F32)
                nc.scalar.activation(
                    out=ee, in_=sp, func=mybir.ActivationFunctionType.Exp, bias=b2_sb
                )

                for b in range(NBPC):
                    st = st_pool.tile([BLK, G_LOC], BF16, tag="st")
                    j = (s * CPS + q) * NBPC + b
                    nc.vector.tensor_scalar(
                        out=st,
                        in0=iota_sb,
                        scalar1=bc_sb[:, j : j + 1],
                        scalar2=ee[:, b : b + 1],
                        op0=mybir.AluOpType.is_equal,
                        op1=mybir.AluOpType.mult,
                    )
                    pool_rhs = (
                        xa[:, q * NBPC + b, :]
                        if ablate != "no_xaug"
                        else xt[:, 0, 0 : HIDDEN + 1]
                    )
                    nc.tensor.matmul(
                        acc,
                        lhsT=st,
                        rhs=pool_rhs,
                        start=(rep == 0 and s == 0 and q == 0 and b == 0),
                        stop=(
                            rep == repeats - 1
                            and s == nsup - 1
                            and q == CPS - 1
                            and b == NBPC - 1
                        ),
                    )

        denom = singles.tile([G_LOC, 1], F32)
        nc.vector.tensor_scalar_max(out=denom, in0=acc[:, HIDDEN : HIDDEN + 1], scalar1=1e-30)
        rdenom = singles.tile([G_LOC, 1], F32)
        nc.vector.reciprocal(out=rdenom, in_=denom)
        out_sb = singles.tile([G_LOC, HIDDEN], F32)
        nc.vector.tensor_scalar_mul(out=out_sb, in0=acc[:, 0:HIDDEN], scalar1=rdenom)
        nc.sync.dma_start(out=out[:, :], in_=out_sb)

    nc.finalize()
    return nc


def make_in_maps(x, batch, W1, b1, W2, b2):
    """Shard by graph (128 contiguous graphs per core), pad node counts to a
    common multiple of CH, and lay out the per-core device arrays."""
    x = np.asarray(x, dtype=np.float32)
    batch = np.asarray(batch)
    bounds = np.searchsorted(batch, np.arange(0, NUM_GRAPHS + 1, G_LOC))
    n_loc_max = int(np.diff(bounds).max())
    n_pad = max(SUP, ((n_loc_max + SUP - 1) // SUP) * SUP)

    w1_bf = np.asarray(W1, np.float32).astype(ml_dtypes.float8_e4m3)
    w2_bf = np.asarray(W2, np.float32).reshape(H, 1).astype(ml_dtypes.bfloat16)
    b1_f = np.asarray(b1, np.float32).reshape(H, 1)
    b2_f = np.full((BLK, 1), np.float32(np.asarray(b2).reshape(-1)[0]), np.float32)

    in_maps = []
    for c in range(N_CORES):
        s, e = int(bounds[c]), int(bounds[c + 1])
        nloc = e - s
        xs = x[s:e]
        nsup = n_pad // SUP
        nb = NBPC * CPS
        xa = np.zeros((n_pad, HIDDEN + 1), ml_dtypes.bfloat16)
        xa[:nloc, :HIDDEN] = xs
        xa[:nloc, HIDDEN] = 1.0
        # [s*SUP + b*BLK + p, f] -> [s, p, b, f]
        xa = np.ascontiguousarray(
            xa.reshape(nsup, nb, BLK, HIDDEN + 1).transpose(0, 2, 1, 3)
        )
        # [s, p, j, n] = x[s*SUP + n, BLK*j + p]
        xT = np.zeros((HIDDEN, n_pad), ml_dtypes.float8_e4m3)
        xT[:, :nloc] = xs.T.astype(ml_dtypes.float8_e4m3)
        xT = np.ascontiguousarray(
            xT.reshape(2, BLK, nsup, SUP).transpose(2, 1, 0, 3)
        )
        bl = np.full((n_pad,), -1.0, np.float32)
        bl[:nloc] = batch[s:e].astype(np.float32) - np.float32(c * G_LOC)
        bcols = np.ascontiguousarray(bl.reshape(n_pad // BLK, BLK).T)
        in_maps.append(
            {
                "xaug": xa,
                "xT": xT,
                "bcols": bcols,
                "w1": w1_bf,
                "w2": w2_bf,
                "b1": b1_f,
                "b2": b2_f,
            }
        )
    return in_maps, n_pad


def kernel(x, batch, W1, b1, W2, b2):
    from concourse.bass_utils import run_bass_kernel_spmd

    in_maps, n_pad = make_in_maps(x, batch, W1, b1, W2, b2)
    nc = _PROGRAM_CACHE.get(n_pad)
    if nc is None:
        nc = build_program(n_pad)
        _PROGRAM_CACHE[n_pad] = nc
    res = run_bass_kernel_spmd(nc, in_maps, list(range(N_CORES)))
    return np.concatenate([res.results[c]["out"] for c in range(N_CORES)], axis=0)

